# revision 1
# baseline (speedup 1.0000x reference)
"""Trainium2 Bass kernel for the BalSCL/SSL balanced supervised-contrastive loss.

Distribution: data-parallel over the 8192 anchor rows, 1024 rows per core on
8 NeuronCores.  Each core computes a partial loss numerator / denominator and
the host combines the 8 scalar pairs.

Math (restructured from the reference, analytically identical):
  N = 8292 columns (8192 anchors + 100 class centers), all unit-norm.
  The row-max subtraction in the reference cancels analytically, so
    loss_i = log(S_i) - (10/m_i) * Sm_i
  with
    S_i  = sum_{j != i} exp(10 * f_i . g_j) / (cc_j - [lab_j == lab_i])
    Sm_i = sum_{j != i, lab_j == lab_i} f_i . g_j
    m_i  = cc[lab_i] - 1      (number of positive pairs for row i)
  Using the one-hot structure everything reduces to per-class aggregates on
  the tensor engine:
    E[c, i]   = sum_{j in class c} exp(10 * rawT[j, i])     (incl. j == i)
    gsum[c,:] = sum_{j in class c} g_j ;  gath[:, i] = gsum[lab_i, :]
  and the diagonal (j == i) contribution is subtracted analytically using
  ||f_i||^2, re-quantized to bf16 so it matches the bf16-stored exp that
  entered E bit-for-bit.  Per-row gathers over classes are one-hot matmuls;
  1/m comes from a per-class constant vector (no reciprocal needed).  The
  final numerator is sum(conf*ln(S)) - sum(conf*SmT); conf (a 0/1 mask) is
  folded into S' = conf*S + (1-conf) so the Ln activation's accumulator
  yields sum(conf*ln(S)) directly.
"""

import os
import sys

sys.path.insert(0, "/opt/trn_rl_repo")

import numpy as np
import ml_dtypes

import concourse.bass as bass  # noqa: F401
import concourse.bacc as bacc
import concourse.tile as tile
from concourse import mybir
from concourse.bass_utils import run_bass_kernel_spmd

F32 = mybir.dt.float32
BF16 = mybir.dt.bfloat16
BF = ml_dtypes.bfloat16
AF = mybir.ActivationFunctionType
ALU = mybir.AluOpType

B2, C, D = 8192, 100, 128
TEMP = 0.1
N = B2 + C                # 8292
TJ = (N + 127) // 128     # 65 j-tiles
NPAD = TJ * 128           # 8320
CORES = 8
R = B2 // CORES           # 1024 rows per core
CH = 512                  # i-chunk width (one fp32 PSUM bank)
GW = 3                    # j-tiles per exp group (3 PSUM banks)
GROUPS = [(g * GW, min(GW, TJ - g * GW)) for g in range((TJ + GW - 1) // GW)]
N_WARM = 7                # PE warm-up matmuls (HAM un-throttle)

FLAG_LNACC = os.environ.get("KB_LNACC", "1") == "1"
FLAG_ONETAB = os.environ.get("KB_ONETAB", "1") == "1"

_NC_CACHE = {}

# Prefer the combined exp+ln activation-table set so the kernel needs a single
# ACT_TABLE_LOAD instead of an exp-set load plus a mid-stream ln-set reload.
_orig_gat = bacc.get_activation_tables


def _gat_combined(arch):
    tabs = _orig_gat(arch)
    if not FLAG_ONETAB:
        return tabs
    out = {}
    for name, funcs in tabs.items():
        if name in ("exp_and_others", "exp_and_friends", "natural_log"):
            out[name] = set()  # keep position (set ids are positional)
        else:
            out[name] = funcs
    return out


def _build_nc():
    bacc.get_activation_tables = _gat_combined
    try:
        return _build_nc_inner()
    finally:
        bacc.get_activation_tables = _orig_gat


def _build_nc_inner():
    nc = bacc.Bacc()

    fTg = nc.dram_tensor("fTg", [D, NPAD], BF16, kind="ExternalInput")
    fAn = nc.dram_tensor("fAn", [128, TJ * 128], BF16, kind="ExternalInput")
    TAg = nc.dram_tensor("TAg", [128, TJ * C], BF16, kind="ExternalInput")
    fTc = nc.dram_tensor("fTc", [D, R], BF16, kind="ExternalInput")
    tTp = nc.dram_tensor("tTp", [C, R], BF16, kind="ExternalInput")
    W2 = nc.dram_tensor("W2", [C, R], F32, kind="ExternalInput")
    conf = nc.dram_tensor("conf", [1, R], F32, kind="ExternalInput")
    rcc = nc.dram_tensor("rcc", [C, 1], BF16, kind="ExternalInput")
    outd = nc.dram_tensor("out", [1, 2], F32, kind="ExternalOutput")

    with tile.TileContext(nc) as tc:
        with (
            tc.tile_pool(name="consts", bufs=1) as cp,
            tc.tile_pool(name="expp", bufs=6) as ep,
            tc.tile_pool(name="asmp", bufs=2) as am,
            tc.tile_pool(name="rawp", bufs=2, space="PSUM") as rp,
            tc.tile_pool(name="epsp", bufs=1, space="PSUM") as pp,
            tc.tile_pool(name="smp", bufs=1, space="PSUM") as sp,
        ):
            # ------------- input loads (ordered by first hardware use) ------
            s_fTc = cp.tile([D, R], BF16)
            s_fTg = cp.tile([D, NPAD], BF16)
            s_TAg = cp.tile([128, TJ * C], BF16)
            s_fAn = cp.tile([128, TJ * 128], BF16)
            nc.sync.dma_start(out=s_fTc[:, 0:CH], in_=fTc[:, 0:CH])
            nc.sync.dma_start(out=s_fTg[:, 0:1024], in_=fTg[:, 0:1024])
            nc.sync.dma_start(out=s_fTg[:, 1024:2560], in_=fTg[:, 1024:2560])
            nc.sync.dma_start(out=s_fTc[:, CH:R], in_=fTc[:, CH:R])
            s_tTp = cp.tile([C, R], BF16)
            nc.sync.dma_start(out=s_tTp, in_=tTp[:])
            s_rcc = cp.tile([C, 1], BF16)
            nc.sync.dma_start(out=s_rcc, in_=rcc[:])
            nc.sync.dma_start(out=s_TAg[:, 0 : 8 * C], in_=TAg[:, 0 : 8 * C])
            nc.sync.dma_start(out=s_fAn[:, 0:1024], in_=fAn[:, 0:1024])
            nc.sync.dma_start(out=s_fTg[:, 2560 : 36 * 128], in_=fTg[:, 2560 : 36 * 128])
            nc.sync.dma_start(out=s_TAg[:, 8 * C : 36 * C], in_=TAg[:, 8 * C : 36 * C])
            nc.sync.dma_start(out=s_fAn[:, 1024 : 36 * 128], in_=fAn[:, 1024 : 36 * 128])
            nc.sync.dma_start(out=s_fTg[:, 36 * 128 :], in_=fTg[:, 36 * 128 :])
            nc.sync.dma_start(out=s_TAg[:, 36 * C :], in_=TAg[:, 36 * C :])
            nc.sync.dma_start(out=s_fAn[:, 36 * 128 :], in_=fAn[:, 36 * 128 :])
            s_conf = cp.tile([1, R], F32)
            nc.sync.dma_start(out=s_conf, in_=conf[:])
            s_W2 = cp.tile([C, R], F32)
            nc.sync.dma_start(out=s_W2, in_=W2[:])

            s_ones = cp.tile([128, 1], F32)
            nc.vector.memset(s_ones, 1.0)
            s_ones_bf = cp.tile([128, 1], BF16)
            nc.vector.memset(s_ones_bf, 1.0)
            s_nones_bf = cp.tile([128, 1], BF16)
            nc.vector.memset(s_nones_bf, -1.0)

            s_gsum = cp.tile([C, D], BF16)
            s_scr = cp.tile([128, CH], BF16)
            nc.vector.memset(s_scr, 1.0)

            # PE warm-up in the DMA-wait window: HAM un-throttles ~3.4us in
            warmPS = sp.tile([128, CH], F32, name="warmPS", tag="sm")
            for _ in range(8):
                nc.tensor.matmul(
                    warmPS, lhsT=s_scr[:, 0:128], rhs=s_scr, start=True, stop=True
                )

            # conf denominator (off the critical tail)
            denv = am.tile([1, 1], F32)
            nc.vector.reduce_sum(out=denv, in_=s_conf, axis=mybir.AxisListType.X)

            # ------------- EPS-independent smalls (run in the DMA window) ----
            # minv10[i] = 10/(cc[lab_i]-1): exact per-class select
            minv = am.tile([1, R], F32, name="minv", tag="minv")
            for k in (0, 1):
                i0 = k * CH
                mPS = sp.tile([1, CH], F32, name=f"mPS{k}", tag="sm")
                nc.tensor.matmul(
                    mPS, lhsT=s_rcc, rhs=s_tTp[:, i0 : i0 + CH],
                    start=True, stop=True,
                )
                nc.vector.tensor_copy(minv[:, i0 : i0 + CH], mPS)

            # sq (f32, for fsq) + sq_bf (bf16, for the smr colsum)
            sq_bf = am.tile([128, R], BF16, name="sq_bf", tag="sq_bf")
            nc.vector.tensor_mul(sq_bf, s_fTc, s_fTc)
            dg_t = [None, None]
            for k in (0, 1):
                i0 = k * CH
                sq = am.tile([128, CH], F32, name=f"sq{k}", tag="sq")
                nc.vector.tensor_mul(
                    sq, s_fTc[:, i0 : i0 + CH], s_fTc[:, i0 : i0 + CH]
                )
                fsqPS = sp.tile([1, CH], F32, name=f"fsqPS{k}", tag="sm")
                nc.tensor.matmul(fsqPS, lhsT=s_ones, rhs=sq, start=True, stop=True)
                ed_bf = am.tile([1, CH], BF16, name=f"edb{k}", tag="edb")
                nc.scalar.activation(
                    out=ed_bf, in_=fsqPS, func=AF.Exp, scale=1.0 / TEMP
                )
                # dg = exp(10 fsq)/m ; with conf folding:
                #   e1 = (dg + 1)*conf - 1  so that  S' = conf*S + (1-conf)
                dg = am.tile([1, CH], F32, name=f"dg{k}", tag="dg")
                nc.vector.scalar_tensor_tensor(
                    out=dg, in0=ed_bf, scalar=0.1, in1=minv[:, i0 : i0 + CH],
                    op0=ALU.mult, op1=ALU.mult,
                )
                if FLAG_LNACC:
                    e1a = am.tile([1, CH], F32, name=f"e1a{k}", tag="e1a")
                    nc.vector.scalar_tensor_tensor(
                        out=e1a, in0=dg, scalar=1.0, in1=s_conf[:, i0 : i0 + CH],
                        op0=ALU.add, op1=ALU.mult,
                    )
                    e1 = am.tile([1, CH], F32, name=f"e1{k}", tag="e1")
                    nc.vector.tensor_scalar_add(e1, e1a, -1.0)
                    dg_t[k] = e1
                else:
                    dg_t[k] = dg

            # ------------- per-chunk raw/exp/E pipeline -------------
            def chunk_body(k, extras=()):
                i0 = k * CH
                extras = dict(extras)
                EPS = pp.tile([C, CH], F32, name=f"EPS{k}", tag="EPS")
                for gi, (t0, gw) in enumerate(GROUPS):
                    rawPS = rp.tile([128, CH * GW], F32, name="rawPS", tag="raw")
                    for q in range(gw):
                        t = t0 + q
                        nc.tensor.matmul(
                            rawPS[:, CH * q : CH * (q + 1)],
                            lhsT=s_fTg[:, 128 * t : 128 * (t + 1)],
                            rhs=s_fTc[:, i0 : i0 + CH],
                            start=True,
                            stop=True,
                        )
                    exps = ep.tile([128, CH * GW], BF16, name="exps", tag="exps")
                    nc.scalar.activation(
                        out=exps[:, : CH * gw],
                        in_=rawPS[:, : CH * gw],
                        func=AF.Exp,
                        scale=1.0 / TEMP,
                    )
                    for q in range(gw):
                        t = t0 + q
                        nc.tensor.matmul(
                            EPS,
                            lhsT=s_TAg[:, C * t : C * (t + 1)],
                            rhs=exps[:, CH * q : CH * (q + 1)],
                            start=(t == 0),
                            stop=(t == TJ - 1),
                        )
                    for fn in extras.pop(gi, ()):
                        fn()
                for fns in extras.values():
                    for fn in fns:
                        fn()
                return EPS

            # ---------------- gsum (interleaved with the chunks) -------------
            gsumPS = sp.tile([C, D], F32, name="gsumPS", tag="sm")
            gsum_state = {"t": 0}

            def gsum_step(n=2):
                def go():
                    t0 = gsum_state["t"]
                    for t in range(t0, min(t0 + n, TJ)):
                        nc.tensor.matmul(
                            gsumPS,
                            lhsT=s_TAg[:, C * t : C * (t + 1)],
                            rhs=s_fAn[:, 128 * t : 128 * (t + 1)],
                            start=(t == 0),
                            stop=(t == TJ - 1),
                        )
                    gsum_state["t"] = min(t0 + n, TJ)
                return go

            Sall = cp.tile([1, R], F32)

            def mk_srow(k, W2E):
                def go():
                    i0 = k * CH
                    SrowPS = sp.tile([1, CH], F32, name=f"SrowPS{k}", tag="sm")
                    nc.tensor.matmul(
                        SrowPS, lhsT=s_ones_bf[0:C, :], rhs=W2E, start=True, stop=True
                    )
                    if FLAG_LNACC:
                        Scm = am.tile([1, CH], F32, name=f"Scm{k}", tag="Scm")
                        nc.vector.tensor_mul(Scm, SrowPS, s_conf[:, i0 : i0 + CH])
                        nc.vector.tensor_sub(Sall[:, i0 : i0 + CH], Scm, dg_t[k])
                    else:
                        nc.vector.tensor_sub(
                            Sall[:, i0 : i0 + CH], SrowPS, dg_t[k]
                        )
                return go

            # Sm path: one N=1024 gather + fused (Asel - fsq) colsum, then
            # SmT = smr * minv and numB = sum(SmT * conf)
            numB = am.tile([1, 1], F32, name="numB", tag="numB")
            gm_state = {}

            s_SmT = cp.tile([1, R], F32)

            def mk_gath(k):
                def go():
                    i0 = k * CH
                    gathT = sp.tile([D, CH], F32, name=f"gathT{k}", tag="sm")
                    nc.tensor.matmul(
                        gathT, lhsT=s_gsum, rhs=s_tTp[:, i0 : i0 + CH],
                        start=True, stop=True,
                    )
                    gmul = am.tile([128, CH], BF16, name=f"gmul{k}", tag="gmul")
                    nc.vector.tensor_mul(gmul, gathT, s_fTc[:, i0 : i0 + CH])
                    gm_state[k] = gmul
                return go

            def mk_smr(k):
                def go():
                    i0 = k * CH
                    smrPS = sp.tile([1, CH], F32, name=f"smrPS{k}", tag="sm")
                    nc.tensor.matmul(
                        smrPS, lhsT=s_ones_bf, rhs=gm_state[k],
                        start=True, stop=False,
                    )
                    nc.tensor.matmul(
                        smrPS, lhsT=s_nones_bf, rhs=sq_bf[:, i0 : i0 + CH],
                        start=False, stop=True,
                    )
                    nc.vector.tensor_mul(
                        s_SmT[:, i0 : i0 + CH], smrPS, minv[:, i0 : i0 + CH]
                    )
                return go

            def mk_smtc():
                def go():
                    smtc = am.tile([1, R], F32, name="smtc", tag="smtc")
                    nc.vector.tensor_mul(smtc, s_SmT, s_conf)
                    nc.vector.reduce_sum(
                        out=numB, in_=smtc, axis=mybir.AxisListType.X
                    )
                return go

            # chunk 0: 2 gsum matmuls interleaved per group
            extras0 = {gi: [gsum_step(2)] for gi in range(1, 22)}
            EPS0 = chunk_body(0, extras=extras0)

            W2E0 = am.tile([C, CH], BF16, name="W2E0", tag="W2E")
            nc.vector.tensor_mul(W2E0, EPS0, s_W2[:, 0:CH])

            # chunk 1: finish gsum early, then the gsum-dependent smalls
            extras1 = {gi: [gsum_step(2)] for gi in range(1, 11)}
            extras1[11] = [gsum_step(TJ)]
            extras1.setdefault(12, []).append(
                lambda: nc.vector.tensor_copy(s_gsum, gsumPS)
            )
            extras1.setdefault(2, []).append(mk_srow(0, W2E0))
            extras1.setdefault(12, []).append(mk_gath(0))
            extras1.setdefault(13, []).append(mk_gath(1))
            extras1.setdefault(14, []).append(mk_smr(0))
            extras1.setdefault(15, []).append(mk_smr(1))
            extras1.setdefault(16, []).append(mk_smtc())
            EPS1 = chunk_body(1, extras=extras1)

            # ---------------- tail ----------------
            W2E1 = am.tile([C, CH], BF16, name="W2E1", tag="W2E")
            nc.vector.tensor_mul(W2E1, EPS1, s_W2[:, CH : 2 * CH])
            mk_srow(1, W2E1)()

            lg = am.tile([1, R], F32)
            numA = am.tile([1, 1], F32)
            if FLAG_LNACC:
                nc.scalar.activation(out=lg, in_=Sall, func=AF.Ln, accum_out=numA)
            else:
                nc.scalar.activation(out=lg, in_=Sall, func=AF.Ln)
                wrow = am.tile([1, R], F32)
                nc.vector.tensor_mul(wrow, lg, s_conf)
                nc.vector.reduce_sum(out=numA, in_=wrow, axis=mybir.AxisListType.X)
            numv = am.tile([1, 1], F32)
            nc.vector.tensor_sub(numv, numA, numB)
            outsb = am.tile([1, 2], F32)
            nc.vector.tensor_copy(outsb[:, 0:1], numv)
            nc.vector.tensor_copy(outsb[:, 1:2], denv)
            nc.sync.dma_start(out=outd[:], in_=outsb)

    nc.finalize()
    return nc


def _get_nc():
    if "nc" not in _NC_CACHE:
        _NC_CACHE["nc"] = _build_nc()
    return _NC_CACHE["nc"]


def _prep_inputs(centers1, features, targets, conf_mask):
    f32 = np.float32
    features = np.ascontiguousarray(features, dtype=f32)
    centers1 = np.ascontiguousarray(centers1, dtype=f32).reshape(-1, D)
    targets = np.ascontiguousarray(targets, dtype=f32)
    conf_mask = np.ascontiguousarray(conf_mask, dtype=f32)

    feats_all = np.concatenate([features, centers1], axis=0)  # [N, D]
    fa_pad = np.zeros((NPAD, D), dtype=f32)
    fa_pad[:N] = feats_all
    TA = np.concatenate([targets, np.eye(C, dtype=f32)], axis=0)  # [N, C]
    TA_pad = np.zeros((NPAD, C), dtype=f32)
    TA_pad[:N] = TA

    fTg_np = np.ascontiguousarray(fa_pad.T).astype(BF)  # [D, NPAD]
    fAn_np = np.ascontiguousarray(
        fa_pad.reshape(TJ, 128, D).transpose(1, 0, 2).reshape(128, TJ * D)
    ).astype(BF)
    TAg_np = np.ascontiguousarray(
        TA_pad.reshape(TJ, 128, C).transpose(1, 0, 2).reshape(128, TJ * C)
    ).astype(BF)

    cc = targets.sum(axis=0, dtype=np.float64) + 1.0  # [C]
    safe = cc > 1.5
    dcls = np.where(safe, 1.0 / np.maximum(cc - 1.0, 1.0) - 1.0 / cc, 0.0)
    invc = 1.0 / cc
    rcc_np = np.where(safe, 10.0 / np.maximum(cc - 1.0, 1.0), 0.0)
    rcc_np = rcc_np.astype(BF).reshape(C, 1)

    in_maps = []
    for c in range(CORES):
        rows = slice(c * R, (c + 1) * R)
        fTc_np = np.ascontiguousarray(fTg_np[:, c * R : (c + 1) * R])
        tTp_f32 = np.ascontiguousarray(targets[rows].T, dtype=f32)  # [C, R]
        tTp_np = tTp_f32.astype(BF)
        W2_np = (dcls[:, None] * tTp_f32 + invc[:, None]).astype(f32)
        conf_np = np.ascontiguousarray(conf_mask[rows].reshape(1, R), dtype=f32)
        in_maps.append(
            {
                "fTg": fTg_np,
                "fAn": fAn_np,
                "TAg": TAg_np,
                "fTc": fTc_np,
                "tTp": tTp_np,
                "W2": W2_np,
                "conf": conf_np,
                "rcc": rcc_np,
            }
        )
    return in_maps


def _run(centers1, features, targets, conf_mask, trace=False, trace_cores=None):
    in_maps = _prep_inputs(centers1, features, targets, conf_mask)
    nc = _get_nc()
    kwargs = {}
    if trace:
        # NTFF profiling under axon: shim the (absent) antenv.axon_hooks
        # module and skip the artifact bucket upload.
        import types
        import concourse.bass_utils as bass_utils

        if "antenv.axon_hooks" not in sys.modules:
            mod = types.ModuleType("antenv.axon_hooks")
            mod._hook = None

            def set_axon_ntff_profile_hook(h):
                mod._hook = h

            def get_axon_ntff_profile_hook():
                return mod._hook

            mod.set_axon_ntff_profile_hook = set_axon_ntff_profile_hook
            mod.get_axon_ntff_profile_hook = get_axon_ntff_profile_hook
            sys.modules["antenv.axon_hooks"] = mod
            from trn_agent_boot.trn_boot import _ntff_profile_via_ctypes

            set_axon_ntff_profile_hook(
                _ntff_profile_via_ctypes("/opt/axon/libaxon_pjrt.so")
            )
        bass_utils.upload_artifacts = lambda tmpdir: "local://" + tmpdir
        kwargs = {"trace": True}
        if trace_cores is not None:
            kwargs["trace_cores"] = trace_cores
    res = run_bass_kernel_spmd(nc, in_maps, core_ids=list(range(CORES)), **kwargs)
    num = 0.0
    den = 0.0
    for r in res.results:
        num += float(r["out"][0, 0])
        den += float(r["out"][0, 1])
    loss = np.array(num / den, dtype=np.float32)
    return loss, res


def kernel(centers1, features, targets, cls_num_list, conf_mask):
    loss, _ = _run(centers1, features, targets, conf_mask)
    return loss



# revision 6
# speedup vs baseline: 1.3135x; 1.3135x over previous
"""Trainium2 Bass kernel for the BalSCL/SSL balanced supervised-contrastive loss.

Distribution: data-parallel over the 8192 anchor rows, 1024 rows per core on
8 NeuronCores.  Each core DMAs per-partition partial accumulators [128, 4]
(ln-sum, Sm-sum, conf-sum) and the host combines.

v2 restructure vs the 98us baseline (which was ACT-engine bound: 68us of EXP):
  * The exp stream is e5m2 (fp8), produced by BOTH the ACT engine
    (exp -> e5 directly) and the Vector engine (Schraudolph bit-trick:
    e5-bits = int8_rne(raw*SA + SB), one fused tensor_scalar per pair),
    split ~53/47 over j-tile pairs.  SA/SB are runtime inputs (calibration).
  * E[c,i] = sum_{j in c} exp(...) uses fp8 DoubleRow matmuls over j-tile
    PAIRS (2x PE throughput), classes padded to 128.  Same for gsum.
  * The diagonal (j==i) term is subtracted with the engine-exact quantized
    value: dgA = e5(ACT-exp(10*fsq)), dgD = schraudolph(fsq), selected
    per-row by host-provided masks (the diag j-tile's engine is known).
  * All per-row scalar work lives in [128, 8] layout (row index =
    partition*8+col... col b = row//128) so DVE ops cost ~8 cols, not 1024;
    partition reductions are tiny transposed matmuls.
  * Final reduction to scalars happens on the HOST (acc3 [128,4] per core).
"""

import os
import sys

sys.path.insert(0, "/opt/trn_rl_repo")

import numpy as np
import ml_dtypes

import concourse.bass as bass  # noqa: F401
import concourse.bacc as bacc
import concourse.tile as tile
from concourse import mybir
from concourse.bass_utils import run_bass_kernel_spmd

F32 = mybir.dt.float32
BF16 = mybir.dt.bfloat16
E5 = mybir.dt.float8e5
I8 = mybir.dt.int8
AF = mybir.ActivationFunctionType
ALU = mybir.AluOpType
PMDR = mybir.MatmulPerfMode.DoubleRow
BF = ml_dtypes.bfloat16
NE5 = ml_dtypes.float8_e5m2

B2, C, D = 8192, 100, 128
CP = 128                  # classes padded for DoubleRow (lhsT free must be 128)
TEMP = 0.1
N = B2 + C                # 8292
TJ = 65                   # j-tiles of 128
NPAD = TJ * 128           # 8320
NPAD2 = 66 * 128          # 8448 (pair padding; tile 65 all-zero)
PAIRS = 32                # DR pairs = tiles 0..63; tile 64 handled single
CORES = 8
R = B2 // CORES           # 1024
CH = 512                  # i-chunk width
SA = np.float32(10.0 * 4.0 / np.log(2.0))   # Schraudolph scale for e5m2 bits
SB_DEFAULT = float(os.environ.get("KB_SB", "59.75"))
N_ACT_PAIRS = int(os.environ.get("KB_NA", "17"))  # ACT pairs per chunk (of 32)

_NC_CACHE = {}


def build_pattern(nA=N_ACT_PAIRS, nP=PAIRS):
    """Interleaved engine assignment for the 32 j-tile pairs ('A' or 'D')."""
    nD = nP - nA
    keyed = [("A", (k + 0.5) / nA) for k in range(nA)] + [
        ("D", (k + 0.5) / nD) for k in range(nD)
    ]
    keyed.sort(key=lambda x: x[1])
    return [e for e, _ in keyed]


PAT = build_pattern()

# Prefer the combined exp+ln activation-table set: single ACT_TABLE_LOAD.
_orig_gat = bacc.get_activation_tables


def _gat_combined(arch):
    tabs = _orig_gat(arch)
    out = {}
    for name, funcs in tabs.items():
        if name in ("exp_and_others", "exp_and_friends", "natural_log"):
            out[name] = set()
        else:
            out[name] = funcs
    return out


def _build_nc():
    bacc.get_activation_tables = _gat_combined
    try:
        return _build_nc_inner()
    finally:
        bacc.get_activation_tables = _orig_gat


def _build_nc_inner():
    nc = bacc.Bacc()

    fTg = nc.dram_tensor("fTg", [D, NPAD], BF16, kind="ExternalInput")
    fTc = nc.dram_tensor("fTc", [D, R], BF16, kind="ExternalInput")
    TAgp = nc.dram_tensor("TAgp", [128, 33, 2, CP], E5, kind="ExternalInput")
    fAnp = nc.dram_tensor("fAnp", [128, 33, 2, D], E5, kind="ExternalInput")
    tTp = nc.dram_tensor("tTp", [CP, R], BF16, kind="ExternalInput")
    W2 = nc.dram_tensor("W2", [CP, R], F32, kind="ExternalInput")
    confT = nc.dram_tensor("confT", [128, 8], F32, kind="ExternalInput")
    mA = nc.dram_tensor("mA", [128, 8], F32, kind="ExternalInput")
    mD = nc.dram_tensor("mD", [128, 8], F32, kind="ExternalInput")
    rcc = nc.dram_tensor("rcc", [CP, 1], BF16, kind="ExternalInput")
    cal = nc.dram_tensor("cal", [128, 2], F32, kind="ExternalInput")
    outd = nc.dram_tensor("out", [128, 4], F32, kind="ExternalOutput")

    with tile.TileContext(nc) as tc:
        with (
            tc.tile_pool(name="consts", bufs=1) as cp,
            tc.tile_pool(name="expp", bufs=4) as ep,
            tc.tile_pool(name="rawp", bufs=3, space="PSUM") as rp,
            tc.tile_pool(name="epsp", bufs=1, space="PSUM") as pp,
            tc.tile_pool(name="scalp", bufs=1, space="PSUM") as sp,
        ):
            # ---------------- input loads (ordered by first use) ----------
            s_fTc = cp.tile([D, R], BF16)
            nc.sync.dma_start(out=s_fTc, in_=fTc[:])
            s_fTg = cp.tile([D, NPAD], BF16)
            nc.sync.dma_start(out=s_fTg[:, 0:1024], in_=fTg[:, 0:1024])
            s_cal = cp.tile([128, 2], F32)
            nc.sync.dma_start(out=s_cal, in_=cal[:])
            s_rcc = cp.tile([CP, 1], BF16)
            nc.sync.dma_start(out=s_rcc, in_=rcc[:])
            s_tTp = cp.tile([CP, R], BF16)
            nc.sync.dma_start(out=s_tTp, in_=tTp[:])
            s_TAgp = cp.tile([128, 33, 2, CP], E5)
            nc.sync.dma_start(out=s_TAgp[:, 0:8], in_=TAgp[:, 0:8])
            nc.sync.dma_start(out=s_fTg[:, 1024:3072], in_=fTg[:, 1024:3072])
            nc.sync.dma_start(out=s_TAgp[:, 8:33], in_=TAgp[:, 8:33])
            s_confT = cp.tile([128, 8], F32)
            nc.sync.dma_start(out=s_confT, in_=confT[:])
            nc.sync.dma_start(out=s_fTg[:, 3072:5632], in_=fTg[:, 3072:5632])
            s_fAnp = cp.tile([128, 33, 2, D], E5)
            nc.sync.dma_start(out=s_fAnp[:, 0:17], in_=fAnp[:, 0:17])
            nc.sync.dma_start(out=s_fAnp[:, 17:33], in_=fAnp[:, 17:33])
            nc.sync.dma_start(out=s_fTg[:, 5632:NPAD], in_=fTg[:, 5632:NPAD])
            s_mA = cp.tile([128, 8], F32)
            nc.sync.dma_start(out=s_mA, in_=mA[:])
            s_mD = cp.tile([128, 8], F32)
            nc.sync.dma_start(out=s_mD, in_=mD[:])
            s_W2 = cp.tile([CP, R], F32)
            nc.sync.dma_start(out=s_W2, in_=W2[:])

            # ---------------- constants / persistent aux ------------------
            s_ones_bf = cp.tile([128, 1], BF16)
            nc.vector.memset(s_ones_bf, 1.0)
            s_ones_f = cp.tile([128, 1], F32)
            nc.vector.memset(s_ones_f, 1.0)
            s_nones_f = cp.tile([128, 1], F32)
            nc.vector.memset(s_nones_f, -1.0)
            s_scr = cp.tile([128, 512], BF16)
            nc.vector.memset(s_scr, 1.0)

            s_gsum = cp.tile([CP, D], BF16)
            s_minv = cp.tile([128, 8], F32)
            s_Sall = cp.tile([128, 8], F32)
            s_acc3 = cp.tile([128, 4], F32)
            s_sq = cp.tile([128, CH], F32)
            s_gmul = cp.tile([128, CH], BF16)
            s_W2E = [cp.tile([CP, CH], BF16, name=f"W2E{k}") for k in (0, 1)]
            s_dgA = [cp.tile([128, 4], E5, name=f"dgA{k}") for k in (0, 1)]
            s_dgD = [cp.tile([128, 4], E5, name=f"dgD{k}") for k in (0, 1)]
            s_e1 = [cp.tile([128, 4], F32, name=f"e1{k}") for k in (0, 1)]
            s_SmT = cp.tile([128, 8], F32)

            # scal PSUM bank, hand-sliced (all tiny accumulators)
            scalPS = sp.tile([128, CH], F32, name="scalPS", tag="scal")
            gsumPS = scalPS[:, 128:256]      # [128(C), 128(D)]
            minvT = scalPS[:, 256:264]       # [128, 8]
            fsqT = scalPS[:, 264:272]        # [128, 8]
            smrT = scalPS[:, 272:280]        # [128, 8]
            SrowT = scalPS[:, 280:288]       # [128, 8]

            # ---------------- PE warm-up in the DMA window ----------------
            warmPS = pp.tile([CP, CH], F32, name="warmPS", tag="eps")
            for _ in range(8):
                nc.tensor.matmul(warmPS, lhsT=s_scr[:, 0:128], rhs=s_scr,
                                 start=True, stop=True)

            # ---------------- small helper emitters -----------------------
            def mk_minv(b0, b1):
                def go():
                    for b in range(b0, b1):
                        nc.tensor.matmul(
                            minvT[:, b : b + 1],
                            lhsT=s_tTp[:, 128 * b : 128 * (b + 1)],
                            rhs=s_rcc, start=True, stop=True,
                        )
                return go

            def mk_denv():
                def go():
                    nc.vector.memset(s_acc3, 0.0)
                    nc.vector.reduce_sum(
                        out=s_acc3[:, 2:3], in_=s_confT, axis=mybir.AxisListType.X
                    )
                    nc.vector.tensor_copy(s_minv, minvT)
                return go

            def mk_sq(k):
                def go():
                    i0 = k * CH
                    nc.vector.tensor_mul(
                        s_sq, s_fTc[:, i0 : i0 + CH], s_fTc[:, i0 : i0 + CH]
                    )
                return go

            def mk_fsq(k):
                def go():
                    for b in range(4):
                        nc.tensor.matmul(
                            fsqT[:, 4 * k + b : 4 * k + b + 1],
                            lhsT=s_sq[:, 128 * b : 128 * (b + 1)],
                            rhs=s_ones_f, start=True, stop=True,
                        )
                return go

            def mk_dg(k):
                def go():
                    sl = slice(4 * k, 4 * k + 4)
                    nc.scalar.activation(
                        out=s_dgA[k], in_=fsqT[:, sl], func=AF.Exp, scale=1.0 / TEMP
                    )
                    nc.vector.tensor_scalar(
                        s_dgD[k].bitcast(I8), fsqT[:, sl],
                        s_cal[:, 0:1], s_cal[:, 1:2], op0=ALU.mult, op1=ALU.add,
                    )
                    t0 = cp.tile([128, 4], F32, name=f"dgt0_{k}")
                    nc.vector.tensor_mul(t0, s_dgA[k], s_mA[:, sl])
                    t1 = cp.tile([128, 4], F32, name=f"dgt1_{k}")
                    nc.vector.tensor_mul(t1, s_dgD[k], s_mD[:, sl])
                    dgsel = cp.tile([128, 4], F32, name=f"dgsel{k}")
                    nc.vector.tensor_add(dgsel, t0, t1)
                    dgv = cp.tile([128, 4], F32, name=f"dgv{k}")
                    nc.vector.scalar_tensor_tensor(
                        out=dgv, in0=dgsel, scalar=TEMP, in1=s_minv[:, sl],
                        op0=ALU.mult, op1=ALU.mult,
                    )
                    e1a = cp.tile([128, 4], F32, name=f"e1a{k}")
                    nc.vector.scalar_tensor_tensor(
                        out=e1a, in0=dgv, scalar=1.0, in1=s_confT[:, sl],
                        op0=ALU.add, op1=ALU.mult,
                    )
                    nc.vector.tensor_scalar_add(s_e1[k], e1a, -1.0)
                return go

            def mk_w2e(k, EPS):
                def go():
                    i0 = k * CH
                    nc.vector.tensor_mul(s_W2E[k], EPS, s_W2[:, i0 : i0 + CH])
                return go

            def mk_srow(k):
                def go():
                    for b in range(4):
                        nc.tensor.matmul(
                            SrowT[:, 4 * k + b : 4 * k + b + 1],
                            lhsT=s_W2E[k][:, 128 * b : 128 * (b + 1)],
                            rhs=s_ones_bf, start=True, stop=True,
                        )
                return go

            def mk_sall(k):
                def go():
                    sl = slice(4 * k, 4 * k + 4)
                    scm = cp.tile([128, 4], F32, name=f"scm{k}")
                    nc.vector.tensor_mul(scm, SrowT[:, sl], s_confT[:, sl])
                    nc.vector.tensor_sub(s_Sall[:, sl], scm, s_e1[k])
                return go

            gath_t = [None]

            def mk_gath(k):
                def go():
                    i0 = k * CH
                    gT = rp.tile([128, 2, CH], F32, name=f"gathT{k}", tag="pair")
                    nc.tensor.matmul(
                        gT[:, 0, :], lhsT=s_gsum, rhs=s_tTp[:, i0 : i0 + CH],
                        start=True, stop=True,
                    )
                    gath_t[0] = gT
                return go

            def mk_gmul(k):
                def go():
                    i0 = k * CH
                    nc.vector.tensor_mul(
                        s_gmul, gath_t[0][:, 0, :], s_fTc[:, i0 : i0 + CH]
                    )
                return go

            def mk_smr(k):
                def go():
                    for b in range(4):
                        col = smrT[:, 4 * k + b : 4 * k + b + 1]
                        nc.tensor.matmul(
                            col, lhsT=s_gmul[:, 128 * b : 128 * (b + 1)],
                            rhs=s_ones_bf, start=True, stop=False,
                        )
                        nc.tensor.matmul(
                            col, lhsT=s_sq[:, 128 * b : 128 * (b + 1)],
                            rhs=s_nones_f, start=False, stop=True,
                        )
                return go

            def mk_smt():
                def go():
                    nc.vector.tensor_mul(s_SmT, smrT, s_minv)
                    smtc = cp.tile([128, 8], F32, name="smtc")
                    nc.vector.scalar_tensor_tensor(
                        out=smtc, in0=s_SmT, scalar=1.0, in1=s_confT,
                        op0=ALU.mult, op1=ALU.mult, accum_out=s_acc3[:, 1:2],
                    )
                return go

            gsum_state = {"p": 0}

            def mk_gsum(n):
                def go():
                    p0 = gsum_state["p"]
                    for gp in range(p0, min(p0 + n, 33)):
                        if gp < 32:
                            nc.tensor.matmul(
                                gsumPS, lhsT=s_TAgp[:, gp], rhs=s_fAnp[:, gp],
                                start=(gp == 0), stop=False, perf_mode=PMDR,
                            )
                        else:
                            nc.tensor.matmul(
                                gsumPS, lhsT=s_TAgp[:, 32, 0, :],
                                rhs=s_fAnp[:, 32, 0, :], start=False, stop=True,
                            )
                    gsum_state["p"] = min(p0 + n, 33)
                return go

            def mk_gcopy():
                def go():
                    nc.vector.tensor_copy(s_gsum, gsumPS)
                return go

            # ---------------- main chunk pipeline -------------------------
            def chunk_body(k, extras):
                i0 = k * CH
                EPS = pp.tile([CP, CH], F32, name=f"EPS{k}", tag="eps")
                pend = {}
                for p in range(35):
                    if p <= 32:
                        W = 2 if p < 32 else 1
                        rawPS = rp.tile([128, 2, CH], F32, name="rawPS", tag="pair")
                        for q in range(W):
                            t = 2 * p + q
                            nc.tensor.matmul(
                                rawPS[:, q, :],
                                lhsT=s_fTg[:, 128 * t : 128 * (t + 1)],
                                rhs=s_fTc[:, i0 : i0 + CH],
                                start=True, stop=True,
                            )
                        exps = ep.tile([128, 2, CH], E5, name="exps", tag="exps")
                        eng = PAT[p] if p < 32 else "A"
                        if eng == "A":
                            nc.scalar.activation(
                                out=exps[:, :W, :], in_=rawPS[:, :W, :],
                                func=AF.Exp, scale=1.0 / TEMP,
                            )
                        else:
                            nc.vector.tensor_scalar(
                                exps.bitcast(I8)[:, :W, :], rawPS[:, :W, :],
                                s_cal[:, 0:1], s_cal[:, 1:2],
                                op0=ALU.mult, op1=ALU.add,
                            )
                        pend[p] = (exps, W)
                    if p >= 2 and (p - 2) in pend:
                        exps, W = pend.pop(p - 2)
                        pp_ = p - 2
                        if W == 2:
                            nc.tensor.matmul(
                                EPS, lhsT=s_TAgp[:, pp_], rhs=exps[:],
                                start=(pp_ == 0), stop=False, perf_mode=PMDR,
                            )
                        else:
                            nc.tensor.matmul(
                                EPS, lhsT=s_TAgp[:, 32, 0, :], rhs=exps[:, 0, :],
                                start=False, stop=True,
                            )
                    for fn in extras.get(p, ()):
                        fn()
                return EPS

            extras0 = {
                1: [mk_minv(0, 4)],
                2: [mk_minv(4, 8)],
                3: [mk_denv()],
                5: [mk_sq(0)],
                7: [mk_fsq(0)],
            }
            for p in range(8, 24):
                extras0[p] = [mk_gsum(2)]
            extras0[24] = [mk_gsum(33)]
            EPS0 = chunk_body(0, extras0)

            extras1 = {
                0: [mk_gcopy()],
                1: [mk_w2e(0, EPS0)],
                2: [mk_srow(0)],
                3: [mk_dg(0)],
                4: [mk_sall(0)],
                5: [mk_gath(0)],
                6: [mk_gmul(0)],
                7: [mk_smr(0)],
                8: [mk_sq(1)],
                10: [mk_fsq(1)],
                11: [mk_dg(1)],
                12: [mk_gath(1)],
                13: [mk_gmul(1)],
                14: [mk_smr(1)],
                15: [mk_smt()],
            }
            EPS1 = chunk_body(1, extras1)

            # ---------------- tail ----------------------------------------
            mk_w2e(1, EPS1)()
            mk_srow(1)()
            mk_sall(1)()
            lg = cp.tile([128, 8], F32)
            nc.scalar.activation(
                out=lg, in_=s_Sall, func=AF.Ln, accum_out=s_acc3[:, 0:1]
            )
            nc.sync.dma_start(out=outd[:], in_=s_acc3)

    nc.finalize()
    return nc


def _get_nc():
    if "nc" not in _NC_CACHE:
        _NC_CACHE["nc"] = _build_nc()
    return _NC_CACHE["nc"]


def _prep_inputs(centers1, features, targets, conf_mask, sb=SB_DEFAULT):
    f32 = np.float32
    features = np.ascontiguousarray(features, dtype=f32)
    centers1 = np.ascontiguousarray(centers1, dtype=f32).reshape(-1, D)
    targets = np.ascontiguousarray(targets, dtype=f32)
    conf_mask = np.ascontiguousarray(conf_mask, dtype=f32)

    feats_all = np.concatenate([features, centers1], axis=0)      # [N, D]
    fa_pad = np.zeros((NPAD2, D), dtype=f32)
    fa_pad[:N] = feats_all
    TA_pad = np.zeros((NPAD2, CP), dtype=f32)
    TA_pad[:B2, :C] = targets
    TA_pad[B2:N, :C] = np.eye(C, dtype=f32)

    fTg_np = np.ascontiguousarray(fa_pad[:NPAD].T).astype(BF)     # [D, NPAD]
    TAgp_np = np.ascontiguousarray(
        TA_pad.reshape(33, 2, 128, CP).transpose(2, 0, 1, 3)
    ).astype(NE5)                                                  # [128,33,2,CP]
    fAnp_np = np.ascontiguousarray(
        fa_pad.reshape(33, 2, 128, D).transpose(2, 0, 1, 3)
    ).astype(NE5)                                                  # [128,33,2,D]

    cc = targets.sum(axis=0, dtype=np.float64) + 1.0               # [C]
    safe = cc > 1.5
    dcls = np.where(safe, 1.0 / np.maximum(cc - 1.0, 1.0) - 1.0 / cc, 0.0)
    invc = 1.0 / cc
    rcc_np = np.zeros((CP, 1), f32)
    rcc_np[:C, 0] = np.where(safe, 10.0 / np.maximum(cc - 1.0, 1.0), 0.0)
    rcc_np = rcc_np.astype(BF)

    cal_np = np.zeros((128, 2), f32)
    cal_np[:, 0] = SA
    cal_np[:, 1] = f32(sb)

    labels = targets.argmax(1)

    in_maps = []
    for c in range(CORES):
        rows = slice(c * R, (c + 1) * R)
        fTc_np = np.ascontiguousarray(fTg_np[:, c * R : (c + 1) * R])
        tTp_np = np.zeros((CP, R), f32)
        tTp_np[:C] = targets[rows].T
        tTp_bf = tTp_np.astype(BF)
        W2_np = np.zeros((CP, R), f32)
        W2_np[:C] = (dcls[:, None] * targets[rows].T + invc[:, None]).astype(f32)
        confT_np = np.ascontiguousarray(
            conf_mask[rows].reshape(8, 128).T, dtype=f32
        )
        mA_np = np.zeros((128, 8), f32)
        mD_np = np.zeros((128, 8), f32)
        for b in range(8):
            pair = (8 * c + b) // 2
            if PAT[pair] == "A":
                mA_np[:, b] = 1.0
            else:
                mD_np[:, b] = 1.0
        in_maps.append(
            {
                "fTg": fTg_np, "fTc": fTc_np, "TAgp": TAgp_np, "fAnp": fAnp_np,
                "tTp": tTp_bf, "W2": W2_np, "confT": confT_np,
                "mA": mA_np, "mD": mD_np, "rcc": rcc_np, "cal": cal_np,
            }
        )
    _ = labels
    return in_maps


def _run(centers1, features, targets, conf_mask, trace=False, trace_cores=None,
         sb=SB_DEFAULT):
    in_maps = _prep_inputs(centers1, features, targets, conf_mask, sb=sb)
    nc = _get_nc()
    kwargs = {}
    if trace:
        import types
        import concourse.bass_utils as bass_utils

        if "antenv.axon_hooks" not in sys.modules:
            mod = types.ModuleType("antenv.axon_hooks")
            mod._hook = None

            def set_axon_ntff_profile_hook(h):
                mod._hook = h

            def get_axon_ntff_profile_hook():
                return mod._hook

            mod.set_axon_ntff_profile_hook = set_axon_ntff_profile_hook
            mod.get_axon_ntff_profile_hook = get_axon_ntff_profile_hook
            sys.modules["antenv.axon_hooks"] = mod
            from trn_agent_boot.trn_boot import _ntff_profile_via_ctypes

            set_axon_ntff_profile_hook(
                _ntff_profile_via_ctypes("/opt/axon/libaxon_pjrt.so")
            )
        bass_utils.upload_artifacts = lambda tmpdir: "local://" + tmpdir
        kwargs = {"trace": True}
        if trace_cores is not None:
            kwargs["trace_cores"] = trace_cores
    res = run_bass_kernel_spmd(nc, in_maps, core_ids=list(range(CORES)), **kwargs)
    num = 0.0
    den = 0.0
    for r in res.results:
        acc = np.asarray(r["out"], dtype=np.float64)
        num += acc[:, 0].sum() - acc[:, 1].sum()
        den += acc[:, 2].sum()
    loss = np.array(num / den, dtype=np.float32)
    return loss, res


def kernel(centers1, features, targets, cls_num_list, conf_mask):
    loss, _ = _run(centers1, features, targets, conf_mask)
    return loss
